# revision 13
# baseline (speedup 1.0000x reference)
"""CV neural network (6 modes, cutoff 3, 6 layers) on 8 trn2 NeuronCores.

Algebra: the reference circuit is
    psi0(x_b) = kron_m expm(x_bm * D_GEN)[:, 0]          (closed form, host)
    psi       = C @ psi0                                  (C fixed 729x729)
    out[b,m]  = Re( psi^H (I (x) X_OP (x) I) psi )        (host)
Everything between the data-encoding displacements and the expectations is a
fixed linear operator C on the 729-dim truncated Fock space, depending only on
the (tiny) layer parameters.  The host folds the circuit into UT = C^T once
(complex128), and the device does the only heavy part: the complex matmul
psi[b, i] = sum_j psi0[b, j] * UT[j, i] for 1024 batch samples.

Sharding: batch 4-way x output-column (i) 2-way = 8 cores.  Per core:
  p_re/p_im: [729, 256]  psi0^T batch-quarter (j rows, b cols)
  u_re/u_im: [729, 365]  UT column half (half 1 overlaps one column)
  o_re/o_im: [256, 365]  psi shard (b rows, i cols)
Complex matmul via 4 real matmuls; p_im is negated on-device so both psum
groups are pure '+' accumulations and outputs DMA straight from PSUM.
"""
import os
import numpy as np

N_MODES, N_LAYERS, CUTOFF, BATCH = 6, 6, 3, 1024
M2 = N_MODES * (N_MODES - 1) // 2
DIM = CUTOFF ** N_MODES                      # 729
N_CORES = 8
B_SHARD = BATCH // 4                         # 256 (batch quarter)
I_SHARD = 366                                # even (fp32r needs even N); overlap 3
I_START = (0, DIM - I_SHARD)                 # (0, 363)
J_TILES = [128, 128, 128, 128, 128, 89]      # 729 partition-tiled

MM_F32R = True  # float32r matmul inputs: 1 cyc/row vs fp32's 4 (N>=256)

# Results of the last device run (for the test harness to inspect).
LAST_RESULT = None

# ----------------------------------------------------------------- host math

_a = np.diag(np.sqrt(np.arange(1, CUTOFF)), 1).astype(np.complex128)
_ad = _a.conj().T
_NVEC = np.arange(CUTOFF, dtype=np.float64)
_X_OP = (_a + _ad).real
_BS_GEN = np.kron(_ad, _a) - np.kron(_a, _ad)
_SQ_GEN = _a @ _a - _ad @ _ad
_D_GEN = _ad - _a


def _expm_factory(G):
    """G anti-Hermitian. Returns f(t) = expm(t*G), vectorized over real t."""
    lam, V = np.linalg.eigh(1j * G)
    Vh = V.conj().T

    def f(t):
        t = np.asarray(t, dtype=np.float64)
        ph = np.exp(-1j * np.multiply.outer(t, lam))
        return np.einsum('ij,...j,jk->...ik', V, ph, Vh)
    return f


_disp_gate = _expm_factory(_D_GEN)
_sq_gate_half = _expm_factory(0.5 * _SQ_GEN)
_bs_gate = _expm_factory(_BS_GEN)


def _apply_1(psi, U, m):
    psi = np.moveaxis(psi, 1 + m, -1)
    psi = psi @ U.T
    return np.moveaxis(psi, -1, 1 + m)


def _apply_2(psi, U, m):
    psi = np.moveaxis(psi, (1 + m, 2 + m), (-2, -1))
    sh = psi.shape
    psi = (psi.reshape(sh[:-2] + (CUTOFF * CUTOFF,)) @ U.T).reshape(sh)
    return np.moveaxis(psi, (-2, -1), (1 + m, 2 + m))


def _apply_diag(psi, d, m):
    shape = [1] * psi.ndim
    shape[1 + m] = CUTOFF
    return psi * d.reshape(shape)


def _interferometer(psi, params):
    theta = params[:M2]
    rphi = params[-N_MODES:]
    n = 0
    for l in range(N_MODES):
        for k in range(N_MODES - 1):
            if (l + k) % 2 != 1:
                psi = _apply_2(psi, _bs_gate(theta[n]), k)
                n += 1
    for i in range(max(1, N_MODES - 1)):
        psi = _apply_diag(psi, np.exp(1j * rphi[i] * _NVEC), i)
    return psi


def _build_UT(theta_1, theta_2, squeezing_r, displacement_r, kerr_params):
    """UT[j, i] = C[i, j]: apply the post-encoding circuit to basis vectors."""
    psi = np.eye(DIM, dtype=np.complex128).reshape((DIM,) + (CUTOFF,) * N_MODES)
    for L in range(N_LAYERS):
        psi = _interferometer(psi, theta_1[L])
        for m in range(N_MODES):
            psi = _apply_1(psi, _sq_gate_half(squeezing_r[L, m] * 0.5), m)
        psi = _interferometer(psi, theta_2[L])
        for m in range(N_MODES):
            psi = _apply_1(psi, _disp_gate(displacement_r[L, m]), m)
            psi = _apply_diag(
                psi, np.exp(1j * (kerr_params[L, m] * 0.001) * _NVEC * _NVEC), m)
    return psi.reshape(DIM, DIM)


def _build_psi0(x):
    """x: (B, 6) -> flattened kron of displacement columns, (B, 729)."""
    v = _disp_gate(x)[..., :, 0]
    out = v[:, 0, :]
    for m in range(1, N_MODES):
        out = np.einsum('bi,bj->bij', out, v[:, m, :]).reshape(x.shape[0], -1)
    return out


def _expectation(psi_flat):
    """psi_flat: (B, 729) complex -> (B, 6) float64: <X_m>."""
    B = psi_flat.shape[0]
    outs = []
    for m in range(N_MODES):
        pre, post = CUTOFF ** m, CUTOFF ** (N_MODES - 1 - m)
        psi = psi_flat.reshape(B, pre, CUTOFF, post)
        phi = np.einsum('ij,bpjq->bpiq', _X_OP, psi)
        outs.append(np.sum(psi.conj() * phi, axis=(1, 2, 3)).real)
    return np.stack(outs, axis=1)


# --------------------------------------------------------------- bass kernel

def _build_bass():
    import concourse.mybir as mybir
    import concourse.tile as tile
    from concourse import bacc

    nc = bacc.Bacc("TRN2", target_bir_lowering=False, debug=False,
                   enable_asserts=False, num_devices=N_CORES)
    f32 = mybir.dt.float32
    mdt = mybir.dt.float32r if MM_F32R else f32

    u_re = nc.dram_tensor("u_re", [DIM, I_SHARD], f32, kind="ExternalInput").ap()
    u_im = nc.dram_tensor("u_im", [DIM, I_SHARD], f32, kind="ExternalInput").ap()
    p_re = nc.dram_tensor("p_re", [DIM, B_SHARD], f32, kind="ExternalInput").ap()
    p_im = nc.dram_tensor("p_im", [DIM, B_SHARD], f32, kind="ExternalInput").ap()
    o_re = nc.dram_tensor("o_re", [B_SHARD, I_SHARD], f32, kind="ExternalOutput").ap()
    o_im = nc.dram_tensor("o_im", [B_SHARD, I_SHARD], f32, kind="ExternalOutput").ap()

    def mm(ap):
        return ap

    NJ = len(J_TILES)
    with tile.TileContext(nc) as tc:
        with (
            tc.tile_pool(name="u", bufs=2 * NJ) as u_pool,
            tc.tile_pool(name="p", bufs=3 * NJ) as p_pool,
            tc.tile_pool(name="ps", bufs=4, space="PSUM") as ps_pool,
            tc.tile_pool(name="o", bufs=4) as o_pool,
            tc.tile_pool(name="s", bufs=6) as s_pool,
        ):
            # DMA loads land in f32 staging tiles; VectorE casts them to
            # float32r compute tiles.  This both provides the official
            # "round to FP32r" producer the BIR verifier wants and funnels
            # every matmul's input deps through one engine (PE's fused
            # f32r weight-load slot only fits a single sync wait).
            ur, ui, pr, pi, pn = {}, {}, {}, {}, {}
            for jt in range(NJ):
                kj, j0 = J_TILES[jt], jt * 128
                s_pr = s_pool.tile([128, B_SHARD], f32, tag="spr", name=f"spr{jt}")
                nc.sync.dma_start(out=s_pr[:kj], in_=p_re[j0:j0 + kj])
                pr[jt] = p_pool.tile([128, B_SHARD], mdt, tag="pr", name=f"pr{jt}")
                nc.vector.tensor_copy(out=pr[jt][:kj], in_=s_pr[:kj])

                s_pi = s_pool.tile([128, B_SHARD], f32, tag="spi", name=f"spi{jt}")
                nc.sync.dma_start(out=s_pi[:kj], in_=p_im[j0:j0 + kj])
                pi[jt] = p_pool.tile([128, B_SHARD], mdt, tag="pi", name=f"pi{jt}")
                nc.vector.tensor_copy(out=pi[jt][:kj], in_=s_pi[:kj])
                pn[jt] = p_pool.tile([128, B_SHARD], mdt, tag="pn", name=f"pn{jt}")
                nc.vector.tensor_scalar_mul(pn[jt][:kj], s_pi[:kj], -1.0)

                s_ur = s_pool.tile([128, I_SHARD], f32, tag="sur", name=f"sur{jt}")
                nc.sync.dma_start(out=s_ur[:kj], in_=u_re[j0:j0 + kj])
                ur[jt] = u_pool.tile([128, I_SHARD], mdt, tag="ur", name=f"ur{jt}")
                nc.vector.tensor_copy(out=ur[jt][:kj], in_=s_ur[:kj])

                s_ui = s_pool.tile([128, I_SHARD], f32, tag="sui", name=f"sui{jt}")
                nc.sync.dma_start(out=s_ui[:kj], in_=u_im[j0:j0 + kj])
                ui[jt] = u_pool.tile([128, I_SHARD], mdt, tag="ui", name=f"ui{jt}")
                nc.vector.tensor_copy(out=ui[jt][:kj], in_=s_ui[:kj])

            for bt in range(2):
                bs = slice(bt * 128, (bt + 1) * 128)
                ps_re = ps_pool.tile([128, I_SHARD], f32, tag="psre")
                ps_im = ps_pool.tile([128, I_SHARD], f32, tag="psim")
                for jt in range(NJ):
                    kj = J_TILES[jt]
                    first, last = jt == 0, jt == NJ - 1
                    # re = Pr.Ur + (-Pi).Ui ; im = Pr.Ui + Pi.Ur
                    nc.tensor.matmul(ps_re, mm(pr[jt][:kj, bs]), mm(ur[jt][:kj]),
                                     start=first, stop=False)
                    nc.tensor.matmul(ps_im, mm(pr[jt][:kj, bs]), mm(ui[jt][:kj]),
                                     start=first, stop=False)
                    nc.tensor.matmul(ps_re, mm(pn[jt][:kj, bs]), mm(ui[jt][:kj]),
                                     start=False, stop=last)
                    nc.tensor.matmul(ps_im, mm(pi[jt][:kj, bs]), mm(ur[jt][:kj]),
                                     start=False, stop=last)
                sb_re = o_pool.tile([128, I_SHARD], f32, tag="sbre",
                                    name=f"sbre{bt}")
                sb_im = o_pool.tile([128, I_SHARD], f32, tag="sbim",
                                    name=f"sbim{bt}")
                nc.vector.tensor_copy(out=sb_re, in_=ps_re)
                nc.scalar.copy(out=sb_im, in_=ps_im)
                nc.gpsimd.dma_start(out=o_re[bs], in_=sb_re)
                nc.gpsimd.dma_start(out=o_im[bs], in_=sb_im)
    nc.compile()
    return nc


def kernel(x, theta_1, theta_2, squeezing_r, displacement_r, kerr_params):
    global LAST_RESULT
    x = np.asarray(x, dtype=np.float32)
    UT = _build_UT(np.asarray(theta_1, np.float64), np.asarray(theta_2, np.float64),
                   np.asarray(squeezing_r, np.float64),
                   np.asarray(displacement_r, np.float64),
                   np.asarray(kerr_params, np.float64))
    psi0 = _build_psi0(x.astype(np.float64))          # (B, 729) complex128
    p_t = psi0.T                                      # (729, B)

    in_maps = []
    for c in range(N_CORES):
        q, h = divmod(c, 2)
        bsl = slice(q * B_SHARD, (q + 1) * B_SHARD)
        isl = slice(I_START[h], I_START[h] + I_SHARD)
        in_maps.append({
            "u_re": np.ascontiguousarray(UT.real[:, isl], np.float32),
            "u_im": np.ascontiguousarray(UT.imag[:, isl], np.float32),
            "p_re": np.ascontiguousarray(p_t.real[:, bsl], np.float32),
            "p_im": np.ascontiguousarray(p_t.imag[:, bsl], np.float32),
        })

    # bass_utils' trace path does `from antenv.axon_hooks import ...`
    # unguarded; this image's antenv lacks that module.  Provide a stub so
    # tracing degrades gracefully instead of crashing (e.g. if BASS_TRACE=1).
    try:
        import antenv.axon_hooks  # noqa: F401
    except ImportError:
        import sys
        import types
        stub = types.ModuleType("antenv.axon_hooks")
        stub._hook = None
        stub.set_axon_ntff_profile_hook = lambda h: setattr(stub, "_hook", h)
        stub.get_axon_ntff_profile_hook = lambda: stub._hook
        sys.modules["antenv.axon_hooks"] = stub

    from concourse.bass_utils import run_bass_kernel_spmd
    nc = _build_bass()
    res = run_bass_kernel_spmd(nc, in_maps, core_ids=list(range(N_CORES)),
                               trace=bool(int(os.environ.get("KERNEL_TRACE", "0"))))
    LAST_RESULT = res

    psi = np.empty((BATCH, DIM), dtype=np.complex128)
    for c in range(N_CORES):
        q, h = divmod(c, 2)
        o = res.results[c]
        sh = o["o_re"].astype(np.float64) + 1j * o["o_im"].astype(np.float64)
        bsl = slice(q * B_SHARD, (q + 1) * B_SHARD)
        if h == 0:
            psi[bsl, 0:I_SHARD] = sh
        else:
            psi[bsl, I_SHARD:DIM] = sh[:, I_SHARD - (DIM - I_SHARD):]
    return _expectation(psi).astype(np.float32)


# revision 16
# speedup vs baseline: 1.3608x; 1.3608x over previous
"""CV neural network (6 modes, cutoff 3, 6 layers) on 8 trn2 NeuronCores.

Algebra: the reference circuit is
    psi0(x_b) = kron_m expm(x_bm * D_GEN)[:, 0]          (closed form, host)
    psi       = C @ psi0                                  (C fixed 729x729)
    out[b,m]  = Re( psi^H (I (x) X_OP (x) I) psi )        (host)
Everything between the data-encoding displacements and the expectations is a
fixed linear operator C on the 729-dim truncated Fock space, depending only on
the (tiny) layer parameters.  The host folds the circuit into UT = C^T once
(complex128), and the device does the only heavy part: the complex matmul
psi[b, i] = sum_j psi0[b, j] * UT[j, i] for 1024 batch samples.

Sharding: batch 4-way x output-column (i) 2-way = 8 cores.  Per core:
  p_re/p_im: [729, 256]  psi0^T batch-quarter (j rows, b cols)
  u_re/u_im: [729, 365]  UT column half (half 1 overlaps one column)
  o_re/o_im: [256, 365]  psi shard (b rows, i cols)
Complex matmul via 4 real matmuls; p_im is negated on-device so both psum
groups are pure '+' accumulations and outputs DMA straight from PSUM.
"""
import os
import numpy as np

N_MODES, N_LAYERS, CUTOFF, BATCH = 6, 6, 3, 1024
M2 = N_MODES * (N_MODES - 1) // 2
DIM = CUTOFF ** N_MODES                      # 729
N_CORES = 8
B_SHARD = BATCH // 4                         # 256 (batch quarter)
I_SHARD = 366                                # even (fp32r needs even N); overlap 3
I_START = (0, DIM - I_SHARD)                 # (0, 363)
DIM_PAD = 768                                # 6 x 128 (rows 729.. are zero)
NJ = 6                                       # j tiles, all K=128 after padding

MM_F32R = True  # float32r matmul inputs: 1 cyc/row vs fp32's 4 (N>=256)

# Results of the last device run (for the test harness to inspect).
LAST_RESULT = None

# ----------------------------------------------------------------- host math

_a = np.diag(np.sqrt(np.arange(1, CUTOFF)), 1).astype(np.complex128)
_ad = _a.conj().T
_NVEC = np.arange(CUTOFF, dtype=np.float64)
_X_OP = (_a + _ad).real
_BS_GEN = np.kron(_ad, _a) - np.kron(_a, _ad)
_SQ_GEN = _a @ _a - _ad @ _ad
_D_GEN = _ad - _a


def _expm_factory(G):
    """G anti-Hermitian. Returns f(t) = expm(t*G), vectorized over real t."""
    lam, V = np.linalg.eigh(1j * G)
    Vh = V.conj().T

    def f(t):
        t = np.asarray(t, dtype=np.float64)
        ph = np.exp(-1j * np.multiply.outer(t, lam))
        return np.einsum('ij,...j,jk->...ik', V, ph, Vh)
    return f


_disp_gate = _expm_factory(_D_GEN)
_sq_gate_half = _expm_factory(0.5 * _SQ_GEN)
_bs_gate = _expm_factory(_BS_GEN)


def _apply_1(psi, U, m):
    psi = np.moveaxis(psi, 1 + m, -1)
    psi = psi @ U.T
    return np.moveaxis(psi, -1, 1 + m)


def _apply_2(psi, U, m):
    psi = np.moveaxis(psi, (1 + m, 2 + m), (-2, -1))
    sh = psi.shape
    psi = (psi.reshape(sh[:-2] + (CUTOFF * CUTOFF,)) @ U.T).reshape(sh)
    return np.moveaxis(psi, (-2, -1), (1 + m, 2 + m))


def _apply_diag(psi, d, m):
    shape = [1] * psi.ndim
    shape[1 + m] = CUTOFF
    return psi * d.reshape(shape)


def _interferometer(psi, params):
    theta = params[:M2]
    rphi = params[-N_MODES:]
    n = 0
    for l in range(N_MODES):
        for k in range(N_MODES - 1):
            if (l + k) % 2 != 1:
                psi = _apply_2(psi, _bs_gate(theta[n]), k)
                n += 1
    for i in range(max(1, N_MODES - 1)):
        psi = _apply_diag(psi, np.exp(1j * rphi[i] * _NVEC), i)
    return psi


def _build_UT(theta_1, theta_2, squeezing_r, displacement_r, kerr_params):
    """UT[j, i] = C[i, j]: apply the post-encoding circuit to basis vectors."""
    psi = np.eye(DIM, dtype=np.complex128).reshape((DIM,) + (CUTOFF,) * N_MODES)
    for L in range(N_LAYERS):
        psi = _interferometer(psi, theta_1[L])
        for m in range(N_MODES):
            psi = _apply_1(psi, _sq_gate_half(squeezing_r[L, m] * 0.5), m)
        psi = _interferometer(psi, theta_2[L])
        for m in range(N_MODES):
            psi = _apply_1(psi, _disp_gate(displacement_r[L, m]), m)
            psi = _apply_diag(
                psi, np.exp(1j * (kerr_params[L, m] * 0.001) * _NVEC * _NVEC), m)
    return psi.reshape(DIM, DIM)


def _build_psi0(x):
    """x: (B, 6) -> flattened kron of displacement columns, (B, 729)."""
    v = _disp_gate(x)[..., :, 0]
    out = v[:, 0, :]
    for m in range(1, N_MODES):
        out = np.einsum('bi,bj->bij', out, v[:, m, :]).reshape(x.shape[0], -1)
    return out


def _expectation(psi_flat):
    """psi_flat: (B, 729) complex -> (B, 6) float64: <X_m>."""
    B = psi_flat.shape[0]
    outs = []
    for m in range(N_MODES):
        pre, post = CUTOFF ** m, CUTOFF ** (N_MODES - 1 - m)
        psi = psi_flat.reshape(B, pre, CUTOFF, post)
        phi = np.einsum('ij,bpjq->bpiq', _X_OP, psi)
        outs.append(np.sum(psi.conj() * phi, axis=(1, 2, 3)).real)
    return np.stack(outs, axis=1)


# --------------------------------------------------------------- bass kernel

def _build_bass():
    import concourse.mybir as mybir
    import concourse.tile as tile
    from concourse import bacc

    nc = bacc.Bacc("TRN2", target_bir_lowering=False, debug=False,
                   enable_asserts=False, num_devices=N_CORES)
    f32 = mybir.dt.float32
    mdt = mybir.dt.float32r if MM_F32R else f32

    u_re = nc.dram_tensor("u_re", [DIM_PAD, I_SHARD], f32, kind="ExternalInput").ap()
    u_im = nc.dram_tensor("u_im", [DIM_PAD, I_SHARD], f32, kind="ExternalInput").ap()
    p_re = nc.dram_tensor("p_re", [DIM_PAD, B_SHARD], f32, kind="ExternalInput").ap()
    p_im = nc.dram_tensor("p_im", [DIM_PAD, B_SHARD], f32, kind="ExternalInput").ap()
    o_re = nc.dram_tensor("o_re", [B_SHARD, I_SHARD], f32, kind="ExternalOutput").ap()
    o_im = nc.dram_tensor("o_im", [B_SHARD, I_SHARD], f32, kind="ExternalOutput").ap()

    def mm(ap):
        return ap

    UW, PW = 3 * I_SHARD, 3 * B_SHARD        # per-half fused tile widths
    u_view = [u_re.rearrange("(a p) i -> p a i", p=128),
              u_im.rearrange("(a p) i -> p a i", p=128)]
    p_view = [p_re.rearrange("(a p) b -> p a b", p=128),
              p_im.rearrange("(a p) b -> p a b", p=128)]

    with tile.TileContext(nc) as tc:
        with (
            tc.tile_pool(name="u", bufs=2) as u_pool,
            tc.tile_pool(name="p", bufs=2) as p_pool,
            tc.tile_pool(name="ps", bufs=4, space="PSUM") as ps_pool,
            tc.tile_pool(name="o", bufs=2) as o_pool,
            tc.tile_pool(name="s", bufs=2) as s_pool,
        ):
            # Big fused DMAs (j padded to 768 = 6x128): one [128, 3*W] tile
            # per (tensor, half), spread across the three DGE rings
            # (sync HWDGE / scalar HWDGE / gpsimd SWDGE) so transfers run
            # concurrently.  f32 staging -> VectorE cast to float32r tiles
            # (the rounding producer fp32r matmuls require; also keeps every
            # matmul's input deps on one engine -> single sync wait).
            ur, ui, pr, pi, pn = {}, {}, {}, {}, {}
            for h in range(2):
                s_pr = s_pool.tile([128, PW], f32, tag="spr", name=f"spr{h}")
                nc.gpsimd.dma_start(out=s_pr.rearrange("p (a b) -> p a b", a=3),
                    in_=p_view[0][:, h * 3:(h + 1) * 3, :])
                pr[h] = p_pool.tile([128, PW], mdt, tag="pr", name=f"pr{h}")
                nc.vector.tensor_copy(out=pr[h], in_=s_pr)

                s_pi = s_pool.tile([128, PW], f32, tag="spi", name=f"spi{h}")
                nc.gpsimd.dma_start(out=s_pi.rearrange("p (a b) -> p a b", a=3),
                    in_=p_view[1][:, h * 3:(h + 1) * 3, :])
                pi[h] = p_pool.tile([128, PW], mdt, tag="pi", name=f"pi{h}")
                nc.vector.tensor_copy(out=pi[h], in_=s_pi)
                pn[h] = p_pool.tile([128, PW], mdt, tag="pn", name=f"pn{h}")
                nc.vector.tensor_scalar_mul(pn[h], s_pi, -1.0)

                s_ur = s_pool.tile([128, UW], f32, tag="sur", name=f"sur{h}")
                nc.sync.dma_start(out=s_ur.rearrange("p (a i) -> p a i", a=3),
                  in_=u_view[0][:, h * 3:(h + 1) * 3, :])
                ur[h] = u_pool.tile([128, UW], mdt, tag="ur", name=f"ur{h}")
                nc.vector.tensor_copy(out=ur[h], in_=s_ur)

                s_ui = s_pool.tile([128, UW], f32, tag="sui", name=f"sui{h}")
                nc.scalar.dma_start(out=s_ui.rearrange("p (a i) -> p a i", a=3),
                    in_=u_view[1][:, h * 3:(h + 1) * 3, :])
                ui[h] = u_pool.tile([128, UW], mdt, tag="ui", name=f"ui{h}")
                nc.vector.tensor_copy(out=ui[h], in_=s_ui)

            for bt in range(2):
                ps_re = ps_pool.tile([128, I_SHARD], f32, tag="psre")
                ps_im = ps_pool.tile([128, I_SHARD], f32, tag="psim")
                for jt in range(NJ):
                    h, blk = divmod(jt, 3)
                    us = slice(blk * I_SHARD, (blk + 1) * I_SHARD)
                    bs = slice(blk * B_SHARD + bt * 128,
                               blk * B_SHARD + bt * 128 + 128)
                    first, last = jt == 0, jt == NJ - 1
                    # re = Pr.Ur + (-Pi).Ui ; im = Pr.Ui + Pi.Ur
                    nc.tensor.matmul(ps_re, pr[h][:, bs], ur[h][:, us],
                                     start=first, stop=False)
                    nc.tensor.matmul(ps_im, pr[h][:, bs], ui[h][:, us],
                                     start=first, stop=False)
                    nc.tensor.matmul(ps_re, pn[h][:, bs], ui[h][:, us],
                                     start=False, stop=last)
                    nc.tensor.matmul(ps_im, pi[h][:, bs], ur[h][:, us],
                                     start=False, stop=last)
                sb_re = o_pool.tile([128, I_SHARD], f32, tag="sbre",
                                    name=f"sbre{bt}")
                sb_im = o_pool.tile([128, I_SHARD], f32, tag="sbim",
                                    name=f"sbim{bt}")
                nc.vector.tensor_copy(out=sb_re, in_=ps_re)
                nc.scalar.copy(out=sb_im, in_=ps_im)
                bs_o = slice(bt * 128, (bt + 1) * 128)
                nc.sync.dma_start(out=o_re[bs_o], in_=sb_re)
                nc.scalar.dma_start(out=o_im[bs_o], in_=sb_im)
    nc.compile()
    return nc


def kernel(x, theta_1, theta_2, squeezing_r, displacement_r, kerr_params):
    global LAST_RESULT
    x = np.asarray(x, dtype=np.float32)
    UT = _build_UT(np.asarray(theta_1, np.float64), np.asarray(theta_2, np.float64),
                   np.asarray(squeezing_r, np.float64),
                   np.asarray(displacement_r, np.float64),
                   np.asarray(kerr_params, np.float64))
    psi0 = _build_psi0(x.astype(np.float64))          # (B, 729) complex128
    p_t = psi0.T                                      # (729, B)

    UT_pad = np.zeros((DIM_PAD, DIM), np.complex128)
    UT_pad[:DIM] = UT
    p_pad = np.zeros((DIM_PAD, BATCH), np.complex128)
    p_pad[:DIM] = p_t

    in_maps = []
    for c in range(N_CORES):
        q, h = divmod(c, 2)
        bsl = slice(q * B_SHARD, (q + 1) * B_SHARD)
        isl = slice(I_START[h], I_START[h] + I_SHARD)
        in_maps.append({
            "u_re": np.ascontiguousarray(UT_pad.real[:, isl], np.float32),
            "u_im": np.ascontiguousarray(UT_pad.imag[:, isl], np.float32),
            "p_re": np.ascontiguousarray(p_pad.real[:, bsl], np.float32),
            "p_im": np.ascontiguousarray(p_pad.imag[:, bsl], np.float32),
        })

    # bass_utils' trace path does `from antenv.axon_hooks import ...`
    # unguarded; this image's antenv lacks that module.  Provide a stub so
    # tracing degrades gracefully instead of crashing (e.g. if BASS_TRACE=1).
    try:
        import antenv.axon_hooks  # noqa: F401
    except ImportError:
        import sys
        import types
        stub = types.ModuleType("antenv.axon_hooks")
        stub._hook = None
        stub.set_axon_ntff_profile_hook = lambda h: setattr(stub, "_hook", h)
        stub.get_axon_ntff_profile_hook = lambda: stub._hook
        sys.modules["antenv.axon_hooks"] = stub

    from concourse.bass_utils import run_bass_kernel_spmd
    nc = _build_bass()
    res = run_bass_kernel_spmd(nc, in_maps, core_ids=list(range(N_CORES)),
                               trace=bool(int(os.environ.get("KERNEL_TRACE", "0"))))
    LAST_RESULT = res

    psi = np.empty((BATCH, DIM), dtype=np.complex128)
    for c in range(N_CORES):
        q, h = divmod(c, 2)
        o = res.results[c]
        sh = o["o_re"].astype(np.float64) + 1j * o["o_im"].astype(np.float64)
        bsl = slice(q * B_SHARD, (q + 1) * B_SHARD)
        if h == 0:
            psi[bsl, 0:I_SHARD] = sh
        else:
            psi[bsl, I_SHARD:DIM] = sh[:, I_SHARD - (DIM - I_SHARD):]
    return _expectation(psi).astype(np.float32)


# revision 19
# speedup vs baseline: 1.3856x; 1.0182x over previous
"""CV neural network (6 modes, cutoff 3, 6 layers) on 8 trn2 NeuronCores.

Algebra: the reference circuit is
    psi0(x_b) = kron_m expm(x_bm * D_GEN)[:, 0]          (closed form, host)
    psi       = C @ psi0                                  (C fixed 729x729)
    out[b,m]  = Re( psi^H (I (x) X_OP (x) I) psi )        (host)
Everything between the data-encoding displacements and the expectations is a
fixed linear operator C on the 729-dim truncated Fock space, depending only on
the (tiny) layer parameters.  The host folds the circuit into UT = C^T once
(complex128), and the device does the only heavy part: the complex matmul
psi[b, i] = sum_j psi0[b, j] * UT[j, i] for 1024 batch samples.

Sharding: batch 4-way x output-column (i) 2-way = 8 cores.  Per core:
  p_re/p_im: [729, 256]  psi0^T batch-quarter (j rows, b cols)
  u_re/u_im: [729, 365]  UT column half (half 1 overlaps one column)
  o_re/o_im: [256, 365]  psi shard (b rows, i cols)
Complex matmul via 4 real matmuls; p_im is negated on-device so both psum
groups are pure '+' accumulations and outputs DMA straight from PSUM.
"""
import os
import numpy as np

N_MODES, N_LAYERS, CUTOFF, BATCH = 6, 6, 3, 1024
M2 = N_MODES * (N_MODES - 1) // 2
DIM = CUTOFF ** N_MODES                      # 729
N_CORES = 8
B_SHARD = BATCH // 4                         # 256 (batch quarter)
I_SHARD = 366                                # even (fp32r needs even N); overlap 3
I_START = (0, DIM - I_SHARD)                 # (0, 363)
DIM_PAD = 768                                # 6 x 128 (rows 729.. are zero)
NJ = 6                                       # j tiles, all K=128 after padding

MM_F32R = True  # float32r matmul inputs: 1 cyc/row vs fp32's 4 (N>=256)

# Results of the last device run (for the test harness to inspect).
LAST_RESULT = None

# ----------------------------------------------------------------- host math

_a = np.diag(np.sqrt(np.arange(1, CUTOFF)), 1).astype(np.complex128)
_ad = _a.conj().T
_NVEC = np.arange(CUTOFF, dtype=np.float64)
_X_OP = (_a + _ad).real
_BS_GEN = np.kron(_ad, _a) - np.kron(_a, _ad)
_SQ_GEN = _a @ _a - _ad @ _ad
_D_GEN = _ad - _a


def _expm_factory(G):
    """G anti-Hermitian. Returns f(t) = expm(t*G), vectorized over real t."""
    lam, V = np.linalg.eigh(1j * G)
    Vh = V.conj().T

    def f(t):
        t = np.asarray(t, dtype=np.float64)
        ph = np.exp(-1j * np.multiply.outer(t, lam))
        return np.einsum('ij,...j,jk->...ik', V, ph, Vh)
    return f


_disp_gate = _expm_factory(_D_GEN)
_sq_gate_half = _expm_factory(0.5 * _SQ_GEN)
_bs_gate = _expm_factory(_BS_GEN)


def _apply_1(psi, U, m):
    psi = np.moveaxis(psi, 1 + m, -1)
    psi = psi @ U.T
    return np.moveaxis(psi, -1, 1 + m)


def _apply_2(psi, U, m):
    psi = np.moveaxis(psi, (1 + m, 2 + m), (-2, -1))
    sh = psi.shape
    psi = (psi.reshape(sh[:-2] + (CUTOFF * CUTOFF,)) @ U.T).reshape(sh)
    return np.moveaxis(psi, (-2, -1), (1 + m, 2 + m))


def _apply_diag(psi, d, m):
    shape = [1] * psi.ndim
    shape[1 + m] = CUTOFF
    return psi * d.reshape(shape)


def _interferometer(psi, params):
    theta = params[:M2]
    rphi = params[-N_MODES:]
    n = 0
    for l in range(N_MODES):
        for k in range(N_MODES - 1):
            if (l + k) % 2 != 1:
                psi = _apply_2(psi, _bs_gate(theta[n]), k)
                n += 1
    for i in range(max(1, N_MODES - 1)):
        psi = _apply_diag(psi, np.exp(1j * rphi[i] * _NVEC), i)
    return psi


def _build_UT(theta_1, theta_2, squeezing_r, displacement_r, kerr_params):
    """UT[j, i] = C[i, j]: apply the post-encoding circuit to basis vectors."""
    psi = np.eye(DIM, dtype=np.complex128).reshape((DIM,) + (CUTOFF,) * N_MODES)
    for L in range(N_LAYERS):
        psi = _interferometer(psi, theta_1[L])
        for m in range(N_MODES):
            psi = _apply_1(psi, _sq_gate_half(squeezing_r[L, m] * 0.5), m)
        psi = _interferometer(psi, theta_2[L])
        for m in range(N_MODES):
            psi = _apply_1(psi, _disp_gate(displacement_r[L, m]), m)
            psi = _apply_diag(
                psi, np.exp(1j * (kerr_params[L, m] * 0.001) * _NVEC * _NVEC), m)
    return psi.reshape(DIM, DIM)


def _build_psi0(x):
    """x: (B, 6) -> flattened kron of displacement columns, (B, 729)."""
    v = _disp_gate(x)[..., :, 0]
    out = v[:, 0, :]
    for m in range(1, N_MODES):
        out = np.einsum('bi,bj->bij', out, v[:, m, :]).reshape(x.shape[0], -1)
    return out


def _expectation(psi_flat):
    """psi_flat: (B, 729) complex -> (B, 6) float64: <X_m>."""
    B = psi_flat.shape[0]
    outs = []
    for m in range(N_MODES):
        pre, post = CUTOFF ** m, CUTOFF ** (N_MODES - 1 - m)
        psi = psi_flat.reshape(B, pre, CUTOFF, post)
        phi = np.einsum('ij,bpjq->bpiq', _X_OP, psi)
        outs.append(np.sum(psi.conj() * phi, axis=(1, 2, 3)).real)
    return np.stack(outs, axis=1)


# --------------------------------------------------------------- bass kernel

def _build_bass():
    import concourse.mybir as mybir
    import concourse.tile as tile
    from concourse import bacc

    nc = bacc.Bacc("TRN2", target_bir_lowering=False, debug=False,
                   enable_asserts=False, num_devices=N_CORES)
    f32 = mybir.dt.float32
    mdt = mybir.dt.float32r if MM_F32R else f32

    # Inputs are host-pre-tiled to partition-major [128, 6*W] so every DMA is
    # a plain 2D transfer with a long contiguous run per partition.
    u_re = nc.dram_tensor("u_re", [128, 6 * I_SHARD], f32, kind="ExternalInput").ap()
    u_im = nc.dram_tensor("u_im", [128, 6 * I_SHARD], f32, kind="ExternalInput").ap()
    p_re = nc.dram_tensor("p_re", [128, 6 * B_SHARD], f32, kind="ExternalInput").ap()
    p_im = nc.dram_tensor("p_im", [128, 6 * B_SHARD], f32, kind="ExternalInput").ap()
    o_re = nc.dram_tensor("o_re", [B_SHARD, I_SHARD], f32, kind="ExternalOutput").ap()
    o_im = nc.dram_tensor("o_im", [B_SHARD, I_SHARD], f32, kind="ExternalOutput").ap()

    def mm(ap):
        return ap

    UW, PW = 3 * I_SHARD, 3 * B_SHARD        # per-half fused tile widths

    with tile.TileContext(nc) as tc:
        with (
            tc.tile_pool(name="u", bufs=2) as u_pool,
            tc.tile_pool(name="p", bufs=2) as p_pool,
            tc.tile_pool(name="ps", bufs=2, space="PSUM") as ps_pool,
            tc.tile_pool(name="o", bufs=2) as o_pool,
            tc.tile_pool(name="s", bufs=2) as s_pool,
        ):
            # Big fused DMAs (j padded to 768 = 6x128): one [128, 3*W] tile
            # per (tensor, half), spread across the three DGE rings
            # (sync HWDGE / scalar HWDGE / gpsimd SWDGE) so transfers run
            # concurrently.  f32 staging -> VectorE cast to float32r tiles
            # (the rounding producer fp32r matmuls require; also keeps every
            # matmul's input deps on one engine -> single sync wait).
            # PE warm-up: dummy f32r matmuls on a memset tile so the HAM
            # un-throttles (1.2 -> 2.4 GHz) before the real matmuls arrive.
            wsrc0 = s_pool.tile([128, 640], f32, tag="warm0", name="warm0")
            nc.vector.memset(wsrc0[:, :], 0)
            wsrc = s_pool.tile([128, 640], mdt, tag="warm", name="warm")
            nc.vector.tensor_copy(out=wsrc, in_=wsrc0)
            ps_w = ps_pool.tile([128, 512], f32, tag="psw", name="psw", bufs=1)
            for w in range(6):
                nc.tensor.matmul(ps_w, wsrc[:, 0:128], wsrc[:, 128:640],
                                 start=True, stop=True)

            ur, ui, pr, pi, pn = {}, {}, {}, {}, {}
            for h in range(2):
                s_pr = s_pool.tile([128, PW], f32, tag="spr", name=f"spr{h}")
                nc.gpsimd.dma_start(out=s_pr, in_=p_re[:, h * PW:(h + 1) * PW])
                pr[h] = p_pool.tile([128, PW], mdt, tag="pr", name=f"pr{h}")
                nc.vector.tensor_copy(out=pr[h], in_=s_pr)

                s_pi = s_pool.tile([128, PW], f32, tag="spi", name=f"spi{h}")
                nc.gpsimd.dma_start(out=s_pi, in_=p_im[:, h * PW:(h + 1) * PW])
                pi[h] = p_pool.tile([128, PW], mdt, tag="pi", name=f"pi{h}")
                nc.vector.tensor_copy(out=pi[h], in_=s_pi)
                pn[h] = p_pool.tile([128, PW], mdt, tag="pn", name=f"pn{h}")
                nc.vector.tensor_scalar_mul(pn[h], s_pi, -1.0)

                s_ur = s_pool.tile([128, UW], f32, tag="sur", name=f"sur{h}")
                nc.sync.dma_start(out=s_ur, in_=u_re[:, h * UW:(h + 1) * UW])
                ur[h] = u_pool.tile([128, UW], mdt, tag="ur", name=f"ur{h}")
                nc.vector.tensor_copy(out=ur[h], in_=s_ur)

                s_ui = s_pool.tile([128, UW], f32, tag="sui", name=f"sui{h}")
                nc.scalar.dma_start(out=s_ui, in_=u_im[:, h * UW:(h + 1) * UW])
                ui[h] = u_pool.tile([128, UW], mdt, tag="ui", name=f"ui{h}")
                nc.vector.tensor_copy(out=ui[h], in_=s_ui)

            for bt in range(2):
                ps_re = ps_pool.tile([128, I_SHARD], f32, tag="psre")
                ps_im = ps_pool.tile([128, I_SHARD], f32, tag="psim")
                for jt in range(NJ):
                    h, blk = divmod(jt, 3)
                    us = slice(blk * I_SHARD, (blk + 1) * I_SHARD)
                    bs = slice(blk * B_SHARD + bt * 128,
                               blk * B_SHARD + bt * 128 + 128)
                    first, last = jt == 0, jt == NJ - 1
                    # re = Pr.Ur + (-Pi).Ui ; im = Pr.Ui + Pi.Ur
                    nc.tensor.matmul(ps_re, pr[h][:, bs], ur[h][:, us],
                                     start=first, stop=False)
                    nc.tensor.matmul(ps_im, pr[h][:, bs], ui[h][:, us],
                                     start=first, stop=False)
                    nc.tensor.matmul(ps_re, pn[h][:, bs], ui[h][:, us],
                                     start=False, stop=last)
                    nc.tensor.matmul(ps_im, pi[h][:, bs], ur[h][:, us],
                                     start=False, stop=last)
                sb_re = o_pool.tile([128, I_SHARD], f32, tag="sbre",
                                    name=f"sbre{bt}")
                sb_im = o_pool.tile([128, I_SHARD], f32, tag="sbim",
                                    name=f"sbim{bt}")
                nc.vector.tensor_copy(out=sb_re, in_=ps_re)
                nc.scalar.copy(out=sb_im, in_=ps_im)
                bs_o = slice(bt * 128, (bt + 1) * 128)
                nc.sync.dma_start(out=o_re[bs_o], in_=sb_re)
                nc.scalar.dma_start(out=o_im[bs_o], in_=sb_im)
    nc.compile()
    return nc


def kernel(x, theta_1, theta_2, squeezing_r, displacement_r, kerr_params):
    global LAST_RESULT
    x = np.asarray(x, dtype=np.float32)
    UT = _build_UT(np.asarray(theta_1, np.float64), np.asarray(theta_2, np.float64),
                   np.asarray(squeezing_r, np.float64),
                   np.asarray(displacement_r, np.float64),
                   np.asarray(kerr_params, np.float64))
    psi0 = _build_psi0(x.astype(np.float64))          # (B, 729) complex128
    p_t = psi0.T                                      # (729, B)

    UT_pad = np.zeros((DIM_PAD, DIM), np.complex128)
    UT_pad[:DIM] = UT
    p_pad = np.zeros((DIM_PAD, BATCH), np.complex128)
    p_pad[:DIM] = p_t

    def tile_pm(arr):
        """[768, W] -> partition-major [128, 6*W] (row j = 128*a + p)."""
        w = arr.shape[1]
        return np.ascontiguousarray(
            arr.reshape(6, 128, w).transpose(1, 0, 2).reshape(128, 6 * w),
            np.float32)

    in_maps = []
    for c in range(N_CORES):
        q, h = divmod(c, 2)
        bsl = slice(q * B_SHARD, (q + 1) * B_SHARD)
        isl = slice(I_START[h], I_START[h] + I_SHARD)
        in_maps.append({
            "u_re": tile_pm(UT_pad.real[:, isl]),
            "u_im": tile_pm(UT_pad.imag[:, isl]),
            "p_re": tile_pm(p_pad.real[:, bsl]),
            "p_im": tile_pm(p_pad.imag[:, bsl]),
        })

    # bass_utils' trace path does `from antenv.axon_hooks import ...`
    # unguarded; this image's antenv lacks that module.  Provide a stub so
    # tracing degrades gracefully instead of crashing (e.g. if BASS_TRACE=1).
    try:
        import antenv.axon_hooks  # noqa: F401
    except ImportError:
        import sys
        import types
        stub = types.ModuleType("antenv.axon_hooks")
        stub._hook = None
        stub.set_axon_ntff_profile_hook = lambda h: setattr(stub, "_hook", h)
        stub.get_axon_ntff_profile_hook = lambda: stub._hook
        sys.modules["antenv.axon_hooks"] = stub

    from concourse.bass_utils import run_bass_kernel_spmd
    nc = _build_bass()
    res = run_bass_kernel_spmd(nc, in_maps, core_ids=list(range(N_CORES)),
                               trace=bool(int(os.environ.get("KERNEL_TRACE", "0"))))
    LAST_RESULT = res

    psi = np.empty((BATCH, DIM), dtype=np.complex128)
    for c in range(N_CORES):
        q, h = divmod(c, 2)
        o = res.results[c]
        sh = o["o_re"].astype(np.float64) + 1j * o["o_im"].astype(np.float64)
        bsl = slice(q * B_SHARD, (q + 1) * B_SHARD)
        if h == 0:
            psi[bsl, 0:I_SHARD] = sh
        else:
            psi[bsl, I_SHARD:DIM] = sh[:, I_SHARD - (DIM - I_SHARD):]
    return _expectation(psi).astype(np.float32)


# revision 20
# speedup vs baseline: 1.3954x; 1.0071x over previous
"""CV neural network (6 modes, cutoff 3, 6 layers) on 8 trn2 NeuronCores.

Algebra: the reference circuit is
    psi0(x_b) = kron_m expm(x_bm * D_GEN)[:, 0]          (closed form, host)
    psi       = C @ psi0                                  (C fixed 729x729)
    out[b,m]  = Re( psi^H (I (x) X_OP (x) I) psi )        (host)
Everything between the data-encoding displacements and the expectations is a
fixed linear operator C on the 729-dim truncated Fock space, depending only on
the (tiny) layer parameters.  The host folds the circuit into UT = C^T once
(complex128), and the device does the only heavy part: the complex matmul
psi[b, i] = sum_j psi0[b, j] * UT[j, i] for 1024 batch samples.

Sharding: batch 4-way x output-column (i) 2-way = 8 cores.  Per core:
  p_re/p_im: [729, 256]  psi0^T batch-quarter (j rows, b cols)
  u_re/u_im: [729, 365]  UT column half (half 1 overlaps one column)
  o_re/o_im: [256, 365]  psi shard (b rows, i cols)
Complex matmul via 4 real matmuls; p_im is negated on-device so both psum
groups are pure '+' accumulations and outputs DMA straight from PSUM.
"""
import os
import numpy as np

N_MODES, N_LAYERS, CUTOFF, BATCH = 6, 6, 3, 1024
M2 = N_MODES * (N_MODES - 1) // 2
DIM = CUTOFF ** N_MODES                      # 729
N_CORES = 8
B_SHARD = BATCH // 4                         # 256 (batch quarter)
I_SHARD = 366                                # even (fp32r needs even N); overlap 3
I_START = (0, DIM - I_SHARD)                 # (0, 363)
DIM_PAD = 768                                # 6 x 128 (rows 729.. are zero)
NJ = 6                                       # j tiles, all K=128 after padding

MM_F32R = True  # float32r matmul inputs: 1 cyc/row vs fp32's 4 (N>=256)

# Results of the last device run (for the test harness to inspect).
LAST_RESULT = None

# ----------------------------------------------------------------- host math

_a = np.diag(np.sqrt(np.arange(1, CUTOFF)), 1).astype(np.complex128)
_ad = _a.conj().T
_NVEC = np.arange(CUTOFF, dtype=np.float64)
_X_OP = (_a + _ad).real
_BS_GEN = np.kron(_ad, _a) - np.kron(_a, _ad)
_SQ_GEN = _a @ _a - _ad @ _ad
_D_GEN = _ad - _a


def _expm_factory(G):
    """G anti-Hermitian. Returns f(t) = expm(t*G), vectorized over real t."""
    lam, V = np.linalg.eigh(1j * G)
    Vh = V.conj().T

    def f(t):
        t = np.asarray(t, dtype=np.float64)
        ph = np.exp(-1j * np.multiply.outer(t, lam))
        return np.einsum('ij,...j,jk->...ik', V, ph, Vh)
    return f


_disp_gate = _expm_factory(_D_GEN)
_sq_gate_half = _expm_factory(0.5 * _SQ_GEN)
_bs_gate = _expm_factory(_BS_GEN)


def _apply_1(psi, U, m):
    psi = np.moveaxis(psi, 1 + m, -1)
    psi = psi @ U.T
    return np.moveaxis(psi, -1, 1 + m)


def _apply_2(psi, U, m):
    psi = np.moveaxis(psi, (1 + m, 2 + m), (-2, -1))
    sh = psi.shape
    psi = (psi.reshape(sh[:-2] + (CUTOFF * CUTOFF,)) @ U.T).reshape(sh)
    return np.moveaxis(psi, (-2, -1), (1 + m, 2 + m))


def _apply_diag(psi, d, m):
    shape = [1] * psi.ndim
    shape[1 + m] = CUTOFF
    return psi * d.reshape(shape)


def _interferometer(psi, params):
    theta = params[:M2]
    rphi = params[-N_MODES:]
    n = 0
    for l in range(N_MODES):
        for k in range(N_MODES - 1):
            if (l + k) % 2 != 1:
                psi = _apply_2(psi, _bs_gate(theta[n]), k)
                n += 1
    for i in range(max(1, N_MODES - 1)):
        psi = _apply_diag(psi, np.exp(1j * rphi[i] * _NVEC), i)
    return psi


def _build_UT(theta_1, theta_2, squeezing_r, displacement_r, kerr_params):
    """UT[j, i] = C[i, j]: apply the post-encoding circuit to basis vectors."""
    psi = np.eye(DIM, dtype=np.complex128).reshape((DIM,) + (CUTOFF,) * N_MODES)
    for L in range(N_LAYERS):
        psi = _interferometer(psi, theta_1[L])
        for m in range(N_MODES):
            psi = _apply_1(psi, _sq_gate_half(squeezing_r[L, m] * 0.5), m)
        psi = _interferometer(psi, theta_2[L])
        for m in range(N_MODES):
            psi = _apply_1(psi, _disp_gate(displacement_r[L, m]), m)
            psi = _apply_diag(
                psi, np.exp(1j * (kerr_params[L, m] * 0.001) * _NVEC * _NVEC), m)
    return psi.reshape(DIM, DIM)


def _build_psi0(x):
    """x: (B, 6) -> flattened kron of displacement columns, (B, 729)."""
    v = _disp_gate(x)[..., :, 0]
    out = v[:, 0, :]
    for m in range(1, N_MODES):
        out = np.einsum('bi,bj->bij', out, v[:, m, :]).reshape(x.shape[0], -1)
    return out


def _expectation(psi_flat):
    """psi_flat: (B, 729) complex -> (B, 6) float64: <X_m>."""
    B = psi_flat.shape[0]
    outs = []
    for m in range(N_MODES):
        pre, post = CUTOFF ** m, CUTOFF ** (N_MODES - 1 - m)
        psi = psi_flat.reshape(B, pre, CUTOFF, post)
        phi = np.einsum('ij,bpjq->bpiq', _X_OP, psi)
        outs.append(np.sum(psi.conj() * phi, axis=(1, 2, 3)).real)
    return np.stack(outs, axis=1)


# --------------------------------------------------------------- bass kernel

def _build_bass():
    import concourse.mybir as mybir
    import concourse.tile as tile
    from concourse import bacc

    nc = bacc.Bacc("TRN2", target_bir_lowering=False, debug=False,
                   enable_asserts=False, num_devices=N_CORES)
    f32 = mybir.dt.float32
    mdt = mybir.dt.float32r if MM_F32R else f32

    # Inputs are host-pre-tiled so each DMA half is one fully-contiguous
    # [128, 3*W] DRAM block (rows 0:128 = half 0, 128:256 = half 1).
    u_re = nc.dram_tensor("u_re", [256, 3 * I_SHARD], f32, kind="ExternalInput").ap()
    u_im = nc.dram_tensor("u_im", [256, 3 * I_SHARD], f32, kind="ExternalInput").ap()
    p_re = nc.dram_tensor("p_re", [256, 3 * B_SHARD], f32, kind="ExternalInput").ap()
    p_im = nc.dram_tensor("p_im", [256, 3 * B_SHARD], f32, kind="ExternalInput").ap()
    o_re = nc.dram_tensor("o_re", [B_SHARD, I_SHARD], f32, kind="ExternalOutput").ap()
    o_im = nc.dram_tensor("o_im", [B_SHARD, I_SHARD], f32, kind="ExternalOutput").ap()

    def mm(ap):
        return ap

    UW, PW = 3 * I_SHARD, 3 * B_SHARD        # per-half fused tile widths

    with tile.TileContext(nc) as tc:
        with (
            tc.tile_pool(name="u", bufs=2) as u_pool,
            tc.tile_pool(name="p", bufs=2) as p_pool,
            tc.tile_pool(name="ps", bufs=2, space="PSUM") as ps_pool,
            tc.tile_pool(name="o", bufs=2) as o_pool,
            tc.tile_pool(name="s", bufs=2) as s_pool,
        ):
            # Big fused DMAs (j padded to 768 = 6x128): one [128, 3*W] tile
            # per (tensor, half), spread across the three DGE rings
            # (sync HWDGE / scalar HWDGE / gpsimd SWDGE) so transfers run
            # concurrently.  f32 staging -> VectorE cast to float32r tiles
            # (the rounding producer fp32r matmuls require; also keeps every
            # matmul's input deps on one engine -> single sync wait).
            # PE warm-up: dummy f32r matmuls on a memset tile so the HAM
            # un-throttles (1.2 -> 2.4 GHz) before the real matmuls arrive.
            wsrc0 = s_pool.tile([128, 640], f32, tag="warm0", name="warm0")
            nc.vector.memset(wsrc0[:, :], 0)
            wsrc = s_pool.tile([128, 640], mdt, tag="warm", name="warm")
            nc.vector.tensor_copy(out=wsrc, in_=wsrc0)
            ps_w = ps_pool.tile([128, 512], f32, tag="psw", name="psw", bufs=1)
            for w in range(6):
                nc.tensor.matmul(ps_w, wsrc[:, 0:128], wsrc[:, 128:640],
                                 start=True, stop=True)

            ur, ui, pr, pi, pn = {}, {}, {}, {}, {}
            for h in range(2):
                s_pr = s_pool.tile([128, PW], f32, tag="spr", name=f"spr{h}")
                nc.gpsimd.dma_start(out=s_pr, in_=p_re[h * 128:(h + 1) * 128])
                pr[h] = p_pool.tile([128, PW], mdt, tag="pr", name=f"pr{h}")
                nc.vector.tensor_copy(out=pr[h], in_=s_pr)

                s_pi = s_pool.tile([128, PW], f32, tag="spi", name=f"spi{h}")
                nc.gpsimd.dma_start(out=s_pi, in_=p_im[h * 128:(h + 1) * 128])
                pi[h] = p_pool.tile([128, PW], mdt, tag="pi", name=f"pi{h}")
                nc.vector.tensor_copy(out=pi[h], in_=s_pi)
                pn[h] = p_pool.tile([128, PW], mdt, tag="pn", name=f"pn{h}")
                nc.vector.tensor_scalar_mul(pn[h], s_pi, -1.0)

                s_ur = s_pool.tile([128, UW], f32, tag="sur", name=f"sur{h}")
                nc.sync.dma_start(out=s_ur, in_=u_re[h * 128:(h + 1) * 128])
                ur[h] = u_pool.tile([128, UW], mdt, tag="ur", name=f"ur{h}")
                nc.vector.tensor_copy(out=ur[h], in_=s_ur)

                s_ui = s_pool.tile([128, UW], f32, tag="sui", name=f"sui{h}")
                nc.scalar.dma_start(out=s_ui, in_=u_im[h * 128:(h + 1) * 128])
                ui[h] = u_pool.tile([128, UW], mdt, tag="ui", name=f"ui{h}")
                nc.vector.tensor_copy(out=ui[h], in_=s_ui)

            for bt in range(2):
                ps_re = ps_pool.tile([128, I_SHARD], f32, tag="psre")
                ps_im = ps_pool.tile([128, I_SHARD], f32, tag="psim")
                for jt in range(NJ):
                    h, blk = divmod(jt, 3)
                    us = slice(blk * I_SHARD, (blk + 1) * I_SHARD)
                    bs = slice(blk * B_SHARD + bt * 128,
                               blk * B_SHARD + bt * 128 + 128)
                    first, last = jt == 0, jt == NJ - 1
                    # re = Pr.Ur + (-Pi).Ui ; im = Pr.Ui + Pi.Ur
                    nc.tensor.matmul(ps_re, pr[h][:, bs], ur[h][:, us],
                                     start=first, stop=False)
                    nc.tensor.matmul(ps_im, pr[h][:, bs], ui[h][:, us],
                                     start=first, stop=False)
                    nc.tensor.matmul(ps_re, pn[h][:, bs], ui[h][:, us],
                                     start=False, stop=last)
                    nc.tensor.matmul(ps_im, pi[h][:, bs], ur[h][:, us],
                                     start=False, stop=last)
                sb_re = o_pool.tile([128, I_SHARD], f32, tag="sbre",
                                    name=f"sbre{bt}")
                sb_im = o_pool.tile([128, I_SHARD], f32, tag="sbim",
                                    name=f"sbim{bt}")
                nc.vector.tensor_copy(out=sb_re, in_=ps_re)
                nc.scalar.copy(out=sb_im, in_=ps_im)
                bs_o = slice(bt * 128, (bt + 1) * 128)
                nc.sync.dma_start(out=o_re[bs_o], in_=sb_re)
                nc.scalar.dma_start(out=o_im[bs_o], in_=sb_im)
    nc.compile()
    return nc


def kernel(x, theta_1, theta_2, squeezing_r, displacement_r, kerr_params):
    global LAST_RESULT
    x = np.asarray(x, dtype=np.float32)
    UT = _build_UT(np.asarray(theta_1, np.float64), np.asarray(theta_2, np.float64),
                   np.asarray(squeezing_r, np.float64),
                   np.asarray(displacement_r, np.float64),
                   np.asarray(kerr_params, np.float64))
    psi0 = _build_psi0(x.astype(np.float64))          # (B, 729) complex128
    p_t = psi0.T                                      # (729, B)

    UT_pad = np.zeros((DIM_PAD, DIM), np.complex128)
    UT_pad[:DIM] = UT
    p_pad = np.zeros((DIM_PAD, BATCH), np.complex128)
    p_pad[:DIM] = p_t

    def tile_pm(arr):
        """[768, W] -> [256, 3*W]: two half-blocks, each partition-major
        [128, 3*W] and fully contiguous in DRAM (row j = 384*h + 128*a + p)."""
        w = arr.shape[1]
        return np.ascontiguousarray(
            arr.reshape(2, 3, 128, w).transpose(0, 2, 1, 3).reshape(256, 3 * w),
            np.float32)

    in_maps = []
    for c in range(N_CORES):
        q, h = divmod(c, 2)
        bsl = slice(q * B_SHARD, (q + 1) * B_SHARD)
        isl = slice(I_START[h], I_START[h] + I_SHARD)
        in_maps.append({
            "u_re": tile_pm(UT_pad.real[:, isl]),
            "u_im": tile_pm(UT_pad.imag[:, isl]),
            "p_re": tile_pm(p_pad.real[:, bsl]),
            "p_im": tile_pm(p_pad.imag[:, bsl]),
        })

    # bass_utils' trace path does `from antenv.axon_hooks import ...`
    # unguarded; this image's antenv lacks that module.  Provide a stub so
    # tracing degrades gracefully instead of crashing (e.g. if BASS_TRACE=1).
    try:
        import antenv.axon_hooks  # noqa: F401
    except ImportError:
        import sys
        import types
        stub = types.ModuleType("antenv.axon_hooks")
        stub._hook = None
        stub.set_axon_ntff_profile_hook = lambda h: setattr(stub, "_hook", h)
        stub.get_axon_ntff_profile_hook = lambda: stub._hook
        sys.modules["antenv.axon_hooks"] = stub

    from concourse.bass_utils import run_bass_kernel_spmd
    nc = _build_bass()
    res = run_bass_kernel_spmd(nc, in_maps, core_ids=list(range(N_CORES)),
                               trace=bool(int(os.environ.get("KERNEL_TRACE", "0"))))
    LAST_RESULT = res

    psi = np.empty((BATCH, DIM), dtype=np.complex128)
    for c in range(N_CORES):
        q, h = divmod(c, 2)
        o = res.results[c]
        sh = o["o_re"].astype(np.float64) + 1j * o["o_im"].astype(np.float64)
        bsl = slice(q * B_SHARD, (q + 1) * B_SHARD)
        if h == 0:
            psi[bsl, 0:I_SHARD] = sh
        else:
            psi[bsl, I_SHARD:DIM] = sh[:, I_SHARD - (DIM - I_SHARD):]
    return _expectation(psi).astype(np.float32)


# revision 23
# speedup vs baseline: 1.4041x; 1.0062x over previous
"""CV neural network (6 modes, cutoff 3, 6 layers) on 8 trn2 NeuronCores.

Algebra: the reference circuit is
    psi0(x_b) = kron_m expm(x_bm * D_GEN)[:, 0]          (closed form, host)
    psi       = C @ psi0                                  (C fixed 729x729)
    out[b,m]  = Re( psi^H (I (x) X_OP (x) I) psi )        (host)
Everything between the data-encoding displacements and the expectations is a
fixed linear operator C on the 729-dim truncated Fock space, depending only on
the (tiny) layer parameters.  The host folds the circuit into UT = C^T once
(complex128), and the device does the only heavy part: the complex matmul
psi[b, i] = sum_j psi0[b, j] * UT[j, i] for 1024 batch samples.

Sharding: batch 4-way x output-column (i) 2-way = 8 cores.  Per core:
  p_re/p_im: [729, 256]  psi0^T batch-quarter (j rows, b cols)
  u_re/u_im: [729, 365]  UT column half (half 1 overlaps one column)
  o_re/o_im: [256, 365]  psi shard (b rows, i cols)
Complex matmul via 4 real matmuls; p_im is negated on-device so both psum
groups are pure '+' accumulations and outputs DMA straight from PSUM.
"""
import os
import numpy as np

N_MODES, N_LAYERS, CUTOFF, BATCH = 6, 6, 3, 1024
M2 = N_MODES * (N_MODES - 1) // 2
DIM = CUTOFF ** N_MODES                      # 729
N_CORES = 8
B_SHARD = BATCH // 4                         # 256 (batch quarter)
I_SHARD = 366                                # even (fp32r needs even N); overlap 3
I_START = (0, DIM - I_SHARD)                 # (0, 363)
DIM_PAD = 768                                # 6 x 128 (rows 729.. are zero)
NJ = 6                                       # j tiles, all K=128 after padding

MM_F32R = True  # float32r matmul inputs: 1 cyc/row vs fp32's 4 (N>=256)

# Results of the last device run (for the test harness to inspect).
LAST_RESULT = None

# ----------------------------------------------------------------- host math

_a = np.diag(np.sqrt(np.arange(1, CUTOFF)), 1).astype(np.complex128)
_ad = _a.conj().T
_NVEC = np.arange(CUTOFF, dtype=np.float64)
_X_OP = (_a + _ad).real
_BS_GEN = np.kron(_ad, _a) - np.kron(_a, _ad)
_SQ_GEN = _a @ _a - _ad @ _ad
_D_GEN = _ad - _a


def _expm_factory(G):
    """G anti-Hermitian. Returns f(t) = expm(t*G), vectorized over real t."""
    lam, V = np.linalg.eigh(1j * G)
    Vh = V.conj().T

    def f(t):
        t = np.asarray(t, dtype=np.float64)
        ph = np.exp(-1j * np.multiply.outer(t, lam))
        return np.einsum('ij,...j,jk->...ik', V, ph, Vh)
    return f


_disp_gate = _expm_factory(_D_GEN)
_sq_gate_half = _expm_factory(0.5 * _SQ_GEN)
_bs_gate = _expm_factory(_BS_GEN)


def _apply_1(psi, U, m):
    psi = np.moveaxis(psi, 1 + m, -1)
    psi = psi @ U.T
    return np.moveaxis(psi, -1, 1 + m)


def _apply_2(psi, U, m):
    psi = np.moveaxis(psi, (1 + m, 2 + m), (-2, -1))
    sh = psi.shape
    psi = (psi.reshape(sh[:-2] + (CUTOFF * CUTOFF,)) @ U.T).reshape(sh)
    return np.moveaxis(psi, (-2, -1), (1 + m, 2 + m))


def _apply_diag(psi, d, m):
    shape = [1] * psi.ndim
    shape[1 + m] = CUTOFF
    return psi * d.reshape(shape)


def _interferometer(psi, params):
    theta = params[:M2]
    rphi = params[-N_MODES:]
    n = 0
    for l in range(N_MODES):
        for k in range(N_MODES - 1):
            if (l + k) % 2 != 1:
                psi = _apply_2(psi, _bs_gate(theta[n]), k)
                n += 1
    for i in range(max(1, N_MODES - 1)):
        psi = _apply_diag(psi, np.exp(1j * rphi[i] * _NVEC), i)
    return psi


def _build_UT(theta_1, theta_2, squeezing_r, displacement_r, kerr_params):
    """UT[j, i] = C[i, j]: apply the post-encoding circuit to basis vectors."""
    psi = np.eye(DIM, dtype=np.complex128).reshape((DIM,) + (CUTOFF,) * N_MODES)
    for L in range(N_LAYERS):
        psi = _interferometer(psi, theta_1[L])
        for m in range(N_MODES):
            psi = _apply_1(psi, _sq_gate_half(squeezing_r[L, m] * 0.5), m)
        psi = _interferometer(psi, theta_2[L])
        for m in range(N_MODES):
            psi = _apply_1(psi, _disp_gate(displacement_r[L, m]), m)
            psi = _apply_diag(
                psi, np.exp(1j * (kerr_params[L, m] * 0.001) * _NVEC * _NVEC), m)
    return psi.reshape(DIM, DIM)


def _build_psi0(x):
    """x: (B, 6) -> flattened kron of displacement columns, (B, 729)."""
    v = _disp_gate(x)[..., :, 0]
    out = v[:, 0, :]
    for m in range(1, N_MODES):
        out = np.einsum('bi,bj->bij', out, v[:, m, :]).reshape(x.shape[0], -1)
    return out


def _expectation(psi_flat):
    """psi_flat: (B, 729) complex -> (B, 6) float64: <X_m>."""
    B = psi_flat.shape[0]
    outs = []
    for m in range(N_MODES):
        pre, post = CUTOFF ** m, CUTOFF ** (N_MODES - 1 - m)
        psi = psi_flat.reshape(B, pre, CUTOFF, post)
        phi = np.einsum('ij,bpjq->bpiq', _X_OP, psi)
        outs.append(np.sum(psi.conj() * phi, axis=(1, 2, 3)).real)
    return np.stack(outs, axis=1)


# --------------------------------------------------------------- bass kernel

def _build_bass():
    import concourse.mybir as mybir
    import concourse.tile as tile
    from concourse import bacc

    nc = bacc.Bacc("TRN2", target_bir_lowering=False, debug=False,
                   enable_asserts=False, num_devices=N_CORES)
    f32 = mybir.dt.float32
    mdt = mybir.dt.float32r if MM_F32R else f32

    bf16 = mybir.dt.bfloat16
    # Inputs host-pre-tiled: halves as contiguous row-blocks [2*128, 3*W]
    # (row = 128*h + p, col = 3W*0.. per-j-block slices within a half).
    u_re = nc.dram_tensor("u_re", [256, 3 * I_SHARD], f32, kind="ExternalInput").ap()
    u_im = nc.dram_tensor("u_im", [256, 3 * I_SHARD], f32, kind="ExternalInput").ap()
    p_re = nc.dram_tensor("p_re", [256, 3 * B_SHARD], f32, kind="ExternalInput").ap()
    p_im = nc.dram_tensor("p_im", [256, 3 * B_SHARD], f32, kind="ExternalInput").ap()
    o_re = nc.dram_tensor("o_re", [B_SHARD, I_SHARD], bf16, kind="ExternalOutput").ap()
    o_im = nc.dram_tensor("o_im", [B_SHARD, I_SHARD], bf16, kind="ExternalOutput").ap()

    UW, PW = 3 * I_SHARD, 3 * B_SHARD
    with tile.TileContext(nc) as tc:
        with (
            tc.tile_pool(name="u", bufs=2) as u_pool,
            tc.tile_pool(name="p", bufs=2) as p_pool,
            tc.tile_pool(name="ps", bufs=2, space="PSUM") as ps_pool,
            tc.tile_pool(name="o", bufs=2) as o_pool,
            tc.tile_pool(name="s", bufs=2) as s_pool,
        ):
            # PE warm-up: dummy f32r matmuls so the HAM un-throttles
            # (1.2 -> 2.4 GHz) before the real matmuls arrive.
            wsrc0 = s_pool.tile([128, 640], f32, tag="warm0", name="warm0", bufs=1)
            nc.vector.memset(wsrc0[:, :], 0)
            wsrc = s_pool.tile([128, 640], mdt, tag="warm", name="warm", bufs=1)
            nc.vector.tensor_copy(out=wsrc, in_=wsrc0)
            ps_w = ps_pool.tile([128, 512], f32, tag="psw", name="psw", bufs=1)
            for w in range(6):
                nc.tensor.matmul(ps_w, wsrc[:, 0:128], wsrc[:, 128:640],
                                 start=True, stop=True)

            # Input loads, balanced across the three DGE rings by measured
            # ring rates (SWDGE ~173 GB/s, each HWDGE ring ~64 GB/s), in
            # consumption order (half 0 first).  f32 staging -> cast to
            # float32r (the rounding producer fp32r matmuls require).
            ur, ui, pr, pi, pn = {}, {}, {}, {}, {}
            s_ui = {}
            for h in range(2):
                rs = slice(h * 128, (h + 1) * 128)
                s_pr = s_pool.tile([128, PW], f32, tag="spr", name=f"spr{h}")
                nc.gpsimd.dma_start(out=s_pr, in_=p_re[rs])
                pr[h] = p_pool.tile([128, PW], mdt, tag="pr", name=f"pr{h}")
                nc.scalar.copy(out=pr[h], in_=s_pr)

                s_pi = s_pool.tile([128, PW], f32, tag="spi", name=f"spi{h}")
                nc.gpsimd.dma_start(out=s_pi, in_=p_im[rs])
                pi[h] = p_pool.tile([128, PW], mdt, tag="pi", name=f"pi{h}")
                nc.scalar.copy(out=pi[h], in_=s_pi)
                pn[h] = p_pool.tile([128, PW], mdt, tag="pn", name=f"pn{h}")
                nc.vector.tensor_scalar_mul(pn[h], s_pi, -1.0)

                s_ur = s_pool.tile([128, UW], f32, tag="sur", name=f"sur{h}")
                ur[h] = u_pool.tile([128, UW], mdt, tag="ur", name=f"ur{h}")
                if h == 0:
                    nc.gpsimd.dma_start(out=s_ur, in_=u_re[rs])
                else:
                    half = (UW // 4) * 2
                    nc.sync.dma_start(out=s_ur[:, :half], in_=u_re[rs, :half])
                    nc.scalar.dma_start(out=s_ur[:, half:], in_=u_re[rs, half:])
                nc.vector.tensor_copy(out=ur[h], in_=s_ur)

                s_ui[h] = s_pool.tile([128, UW], f32, tag="sui", name=f"suih{h}")
                eng = nc.sync if h == 0 else nc.scalar
                eng.dma_start(out=s_ui[h], in_=u_im[h * 128:(h + 1) * 128])
                ui[h] = u_pool.tile([128, UW], mdt, tag="ui", name=f"ui{h}")
                nc.vector.tensor_copy(out=ui[h], in_=s_ui[h])

            ps_re, ps_im = {}, {}
            for bt in range(2):
                ps_re[bt] = ps_pool.tile([128, I_SHARD], f32, tag="psre", name=f"psre{bt}")
                ps_im[bt] = ps_pool.tile([128, I_SHARD], f32, tag="psim", name=f"psim{bt}")
            for jt in range(NJ):
                h, blk = divmod(jt, 3)
                us = slice(blk * I_SHARD, (blk + 1) * I_SHARD)
                for bt in range(2):
                    bs = slice(blk * B_SHARD + bt * 128,
                               blk * B_SHARD + bt * 128 + 128)
                    first, last = jt == 0, jt == NJ - 1
                    # re = Pr.Ur + (-Pi).Ui ; im = Pr.Ui + Pi.Ur
                    nc.tensor.matmul(ps_re[bt], pr[h][:, bs], ur[h][:, us],
                                     start=first, stop=False)
                    nc.tensor.matmul(ps_im[bt], pr[h][:, bs], ui[h][:, us],
                                     start=first, stop=False)
                    nc.tensor.matmul(ps_re[bt], pn[h][:, bs], ui[h][:, us],
                                     start=False, stop=last)
                    nc.tensor.matmul(ps_im[bt], pi[h][:, bs], ur[h][:, us],
                                     start=False, stop=last)
            for bt in range(2):
                bs_o = slice(bt * 128, (bt + 1) * 128)
                sb_re = o_pool.tile([128, I_SHARD], bf16, tag="sbre",
                                    name=f"sbre{bt}")
                sb_im = o_pool.tile([128, I_SHARD], bf16, tag="sbim",
                                    name=f"sbim{bt}")
                nc.vector.tensor_copy(out=sb_re, in_=ps_re[bt])
                nc.scalar.copy(out=sb_im, in_=ps_im[bt])
                nc.sync.dma_start(out=o_re[bs_o], in_=sb_re)
                nc.scalar.dma_start(out=o_im[bs_o], in_=sb_im)
    nc.compile()
    return nc


def kernel(x, theta_1, theta_2, squeezing_r, displacement_r, kerr_params):
    global LAST_RESULT
    x = np.asarray(x, dtype=np.float32)
    UT = _build_UT(np.asarray(theta_1, np.float64), np.asarray(theta_2, np.float64),
                   np.asarray(squeezing_r, np.float64),
                   np.asarray(displacement_r, np.float64),
                   np.asarray(kerr_params, np.float64))
    psi0 = _build_psi0(x.astype(np.float64))          # (B, 729) complex128
    p_t = psi0.T                                      # (729, B)

    UT_pad = np.zeros((DIM_PAD, DIM), np.complex128)
    UT_pad[:DIM] = UT
    p_pad = np.zeros((DIM_PAD, BATCH), np.complex128)
    p_pad[:DIM] = p_t

    def tile_pm(arr):
        """[768, W] -> [256, 3*W]: half-blocks, each [128, 3*W] contiguous."""
        w = arr.shape[1]
        return np.ascontiguousarray(
            arr.reshape(2, 3, 128, w).transpose(0, 2, 1, 3).reshape(256, 3 * w),
            np.float32)

    in_maps = []
    for c in range(N_CORES):
        q, h = divmod(c, 2)
        bsl = slice(q * B_SHARD, (q + 1) * B_SHARD)
        isl = slice(I_START[h], I_START[h] + I_SHARD)
        in_maps.append({
            "u_re": tile_pm(UT_pad.real[:, isl]),
            "u_im": tile_pm(UT_pad.imag[:, isl]),
            "p_re": tile_pm(p_pad.real[:, bsl]),
            "p_im": tile_pm(p_pad.imag[:, bsl]),
        })

    # bass_utils' trace path does `from antenv.axon_hooks import ...`
    # unguarded; this image's antenv lacks that module.  Provide a stub so
    # tracing degrades gracefully instead of crashing (e.g. if BASS_TRACE=1).
    try:
        import antenv.axon_hooks  # noqa: F401
    except ImportError:
        import sys
        import types
        stub = types.ModuleType("antenv.axon_hooks")
        stub._hook = None
        stub.set_axon_ntff_profile_hook = lambda h: setattr(stub, "_hook", h)
        stub.get_axon_ntff_profile_hook = lambda: stub._hook
        sys.modules["antenv.axon_hooks"] = stub

    from concourse.bass_utils import run_bass_kernel_spmd
    nc = _build_bass()
    res = run_bass_kernel_spmd(nc, in_maps, core_ids=list(range(N_CORES)),
                               trace=bool(int(os.environ.get("KERNEL_TRACE", "0"))))
    LAST_RESULT = res

    psi = np.empty((BATCH, DIM), dtype=np.complex128)
    for c in range(N_CORES):
        q, h = divmod(c, 2)
        o = res.results[c]
        sh = (o["o_re"].astype(np.float64)
              + 1j * o["o_im"].astype(np.float64))
        bsl = slice(q * B_SHARD, (q + 1) * B_SHARD)
        if h == 0:
            psi[bsl, 0:I_SHARD] = sh
        else:
            psi[bsl, I_SHARD:DIM] = sh[:, I_SHARD - (DIM - I_SHARD):]
    return _expectation(psi).astype(np.float32)


# revision 24
# speedup vs baseline: 1.4372x; 1.0236x over previous
"""CV neural network (6 modes, cutoff 3, 6 layers) on 8 trn2 NeuronCores.

Algebra: the reference circuit is
    psi0(x_b) = kron_m expm(x_bm * D_GEN)[:, 0]          (closed form, host)
    psi       = C @ psi0                                  (C fixed 729x729)
    out[b,m]  = Re( psi^H (I (x) X_OP (x) I) psi )        (host)
Everything between the data-encoding displacements and the expectations is a
fixed linear operator C on the 729-dim truncated Fock space, depending only on
the (tiny) layer parameters.  The host folds the circuit into UT = C^T once
(complex128), and the device does the only heavy part: the complex matmul
psi[b, i] = sum_j psi0[b, j] * UT[j, i] for 1024 batch samples.

Sharding: batch 4-way x output-column (i) 2-way = 8 cores.  Per core:
  p_re/p_im: [729, 256]  psi0^T batch-quarter (j rows, b cols)
  u_re/u_im: [729, 365]  UT column half (half 1 overlaps one column)
  o_re/o_im: [256, 365]  psi shard (b rows, i cols)
Complex matmul via 4 real matmuls; p_im is negated on-device so both psum
groups are pure '+' accumulations and outputs DMA straight from PSUM.
"""
import os
import numpy as np

N_MODES, N_LAYERS, CUTOFF, BATCH = 6, 6, 3, 1024
M2 = N_MODES * (N_MODES - 1) // 2
DIM = CUTOFF ** N_MODES                      # 729
N_CORES = 8
B_SHARD = BATCH // 4                         # 256 (batch quarter)
I_SHARD = 366                                # even (fp32r needs even N); overlap 3
I_START = (0, DIM - I_SHARD)                 # (0, 363)
DIM_PAD = 768                                # 6 x 128 (rows 729.. are zero)
NJ = 6                                       # j tiles, all K=128 after padding

MM_F32R = True  # float32r matmul inputs: 1 cyc/row vs fp32's 4 (N>=256)

# Results of the last device run (for the test harness to inspect).
LAST_RESULT = None

# ----------------------------------------------------------------- host math

_a = np.diag(np.sqrt(np.arange(1, CUTOFF)), 1).astype(np.complex128)
_ad = _a.conj().T
_NVEC = np.arange(CUTOFF, dtype=np.float64)
_X_OP = (_a + _ad).real
_BS_GEN = np.kron(_ad, _a) - np.kron(_a, _ad)
_SQ_GEN = _a @ _a - _ad @ _ad
_D_GEN = _ad - _a


def _expm_factory(G):
    """G anti-Hermitian. Returns f(t) = expm(t*G), vectorized over real t."""
    lam, V = np.linalg.eigh(1j * G)
    Vh = V.conj().T

    def f(t):
        t = np.asarray(t, dtype=np.float64)
        ph = np.exp(-1j * np.multiply.outer(t, lam))
        return np.einsum('ij,...j,jk->...ik', V, ph, Vh)
    return f


_disp_gate = _expm_factory(_D_GEN)
_sq_gate_half = _expm_factory(0.5 * _SQ_GEN)
_bs_gate = _expm_factory(_BS_GEN)


def _apply_1(psi, U, m):
    psi = np.moveaxis(psi, 1 + m, -1)
    psi = psi @ U.T
    return np.moveaxis(psi, -1, 1 + m)


def _apply_2(psi, U, m):
    psi = np.moveaxis(psi, (1 + m, 2 + m), (-2, -1))
    sh = psi.shape
    psi = (psi.reshape(sh[:-2] + (CUTOFF * CUTOFF,)) @ U.T).reshape(sh)
    return np.moveaxis(psi, (-2, -1), (1 + m, 2 + m))


def _apply_diag(psi, d, m):
    shape = [1] * psi.ndim
    shape[1 + m] = CUTOFF
    return psi * d.reshape(shape)


def _interferometer(psi, params):
    theta = params[:M2]
    rphi = params[-N_MODES:]
    n = 0
    for l in range(N_MODES):
        for k in range(N_MODES - 1):
            if (l + k) % 2 != 1:
                psi = _apply_2(psi, _bs_gate(theta[n]), k)
                n += 1
    for i in range(max(1, N_MODES - 1)):
        psi = _apply_diag(psi, np.exp(1j * rphi[i] * _NVEC), i)
    return psi


def _build_UT(theta_1, theta_2, squeezing_r, displacement_r, kerr_params):
    """UT[j, i] = C[i, j]: apply the post-encoding circuit to basis vectors."""
    psi = np.eye(DIM, dtype=np.complex128).reshape((DIM,) + (CUTOFF,) * N_MODES)
    for L in range(N_LAYERS):
        psi = _interferometer(psi, theta_1[L])
        for m in range(N_MODES):
            psi = _apply_1(psi, _sq_gate_half(squeezing_r[L, m] * 0.5), m)
        psi = _interferometer(psi, theta_2[L])
        for m in range(N_MODES):
            psi = _apply_1(psi, _disp_gate(displacement_r[L, m]), m)
            psi = _apply_diag(
                psi, np.exp(1j * (kerr_params[L, m] * 0.001) * _NVEC * _NVEC), m)
    return psi.reshape(DIM, DIM)


def _build_psi0(x):
    """x: (B, 6) -> flattened kron of displacement columns, (B, 729)."""
    v = _disp_gate(x)[..., :, 0]
    out = v[:, 0, :]
    for m in range(1, N_MODES):
        out = np.einsum('bi,bj->bij', out, v[:, m, :]).reshape(x.shape[0], -1)
    return out


def _expectation(psi_flat):
    """psi_flat: (B, 729) complex -> (B, 6) float64: <X_m>."""
    B = psi_flat.shape[0]
    outs = []
    for m in range(N_MODES):
        pre, post = CUTOFF ** m, CUTOFF ** (N_MODES - 1 - m)
        psi = psi_flat.reshape(B, pre, CUTOFF, post)
        phi = np.einsum('ij,bpjq->bpiq', _X_OP, psi)
        outs.append(np.sum(psi.conj() * phi, axis=(1, 2, 3)).real)
    return np.stack(outs, axis=1)


# --------------------------------------------------------------- bass kernel

def _build_bass():
    import concourse.mybir as mybir
    import concourse.tile as tile
    from concourse import bacc

    nc = bacc.Bacc("TRN2", target_bir_lowering=False, debug=False,
                   enable_asserts=False, num_devices=N_CORES)
    f32 = mybir.dt.float32
    mdt = mybir.dt.float32r if MM_F32R else f32

    bf16 = mybir.dt.bfloat16
    # Inputs host-pre-tiled: halves as contiguous row-blocks [2*128, 3*W].
    # u/p_re go straight into float32r tiles (fp32r's precision loss happens
    # inside the PE; the DMA bits are unchanged) — p_im is staged once so the
    # on-device negate (-Pi) has an official fp32r-rounding producer.
    u_re = nc.dram_tensor("u_re", [256, 3 * I_SHARD], mdt, kind="ExternalInput").ap()
    u_im = nc.dram_tensor("u_im", [256, 3 * I_SHARD], mdt, kind="ExternalInput").ap()
    p_re = nc.dram_tensor("p_re", [256, 3 * B_SHARD], mdt, kind="ExternalInput").ap()
    p_im = nc.dram_tensor("p_im", [256, 3 * B_SHARD], f32, kind="ExternalInput").ap()
    o_re = nc.dram_tensor("o_re", [B_SHARD, I_SHARD], bf16, kind="ExternalOutput").ap()
    o_im = nc.dram_tensor("o_im", [B_SHARD, I_SHARD], bf16, kind="ExternalOutput").ap()

    UW, PW = 3 * I_SHARD, 3 * B_SHARD
    with tile.TileContext(nc) as tc:
        with (
            tc.tile_pool(name="u", bufs=2) as u_pool,
            tc.tile_pool(name="p", bufs=2) as p_pool,
            tc.tile_pool(name="ps", bufs=2, space="PSUM") as ps_pool,
            tc.tile_pool(name="o", bufs=2) as o_pool,
            tc.tile_pool(name="s", bufs=2) as s_pool,
        ):
            # PE warm-up: dummy f32r matmuls bridge until real matmuls start,
            # so the HAM un-throttles (1.2 -> 2.4 GHz) and stays warm.
            wsrc0 = s_pool.tile([128, 640], f32, tag="warm0", name="warm0", bufs=1)
            nc.vector.memset(wsrc0[:, :], 0)
            wsrc = s_pool.tile([128, 640], mdt, tag="warm", name="warm", bufs=1)
            nc.vector.tensor_copy(out=wsrc, in_=wsrc0)
            ps_w = ps_pool.tile([128, 512], f32, tag="psw", name="psw", bufs=1)
            for w in range(12):
                nc.tensor.matmul(ps_w, wsrc[:, 0:128], wsrc[:, 128:640],
                                 start=True, stop=True)

            # Loads: half 0 entirely on the fast SWDGE ring (~270 GB/s); the
            # two slow HWDGE rings (~45 GB/s) prefetch half 1 concurrently.
            ur, ui, pr, pi, pn = {}, {}, {}, {}, {}
            for h in range(2):
                rs = slice(h * 128, (h + 1) * 128)
                ur[h] = u_pool.tile([128, UW], mdt, tag="ur", name=f"ur{h}")
                ui[h] = u_pool.tile([128, UW], mdt, tag="ui", name=f"ui{h}")
                pr[h] = p_pool.tile([128, PW], mdt, tag="pr", name=f"pr{h}")
                s_pi = s_pool.tile([128, PW], f32, tag="spi", name=f"spi{h}")
                pi[h] = p_pool.tile([128, PW], mdt, tag="pi", name=f"pi{h}")
                pn[h] = p_pool.tile([128, PW], mdt, tag="pn", name=f"pn{h}")
                if h == 0:
                    nc.gpsimd.dma_start(out=ur[0], in_=u_re[rs])
                    nc.gpsimd.dma_start(out=pr[0], in_=p_re[rs])
                    nc.gpsimd.dma_start(out=s_pi, in_=p_im[rs])
                    nc.gpsimd.dma_start(out=ui[0], in_=u_im[rs])
                else:
                    nc.sync.dma_start(out=ur[1], in_=u_re[rs])
                    nc.scalar.dma_start(out=ui[1], in_=u_im[rs])
                    nc.gpsimd.dma_start(out=pr[1], in_=p_re[rs])
                    nc.gpsimd.dma_start(out=s_pi, in_=p_im[rs])
                nc.vector.tensor_copy(out=pi[h], in_=s_pi)
                nc.vector.tensor_scalar_mul(pn[h], s_pi, -1.0)

            ps_re, ps_im = {}, {}
            for bt in range(2):
                ps_re[bt] = ps_pool.tile([128, I_SHARD], f32, tag="psre",
                                         name=f"psre{bt}")
                ps_im[bt] = ps_pool.tile([128, I_SHARD], f32, tag="psim",
                                         name=f"psim{bt}")
            for jt in range(NJ):
                h, blk = divmod(jt, 3)
                us = slice(blk * I_SHARD, (blk + 1) * I_SHARD)
                for bt in range(2):
                    bs = slice(blk * B_SHARD + bt * 128,
                               blk * B_SHARD + bt * 128 + 128)
                    first, last = jt == 0, jt == NJ - 1
                    # re = Pr.Ur + (-Pi).Ui ; im = Pr.Ui + Pi.Ur
                    nc.tensor.matmul(ps_re[bt], pr[h][:, bs], ur[h][:, us],
                                     start=first, stop=False)
                    nc.tensor.matmul(ps_im[bt], pr[h][:, bs], ui[h][:, us],
                                     start=first, stop=False)
                    nc.tensor.matmul(ps_re[bt], pn[h][:, bs], ui[h][:, us],
                                     start=False, stop=last)
                    nc.tensor.matmul(ps_im[bt], pi[h][:, bs], ur[h][:, us],
                                     start=False, stop=last)
            for bt in range(2):
                bs_o = slice(bt * 128, (bt + 1) * 128)
                sb_re = o_pool.tile([128, I_SHARD], bf16, tag="sbre",
                                    name=f"sbre{bt}")
                sb_im = o_pool.tile([128, I_SHARD], bf16, tag="sbim",
                                    name=f"sbim{bt}")
                nc.vector.tensor_copy(out=sb_re, in_=ps_re[bt])
                nc.scalar.copy(out=sb_im, in_=ps_im[bt])
                nc.gpsimd.dma_start(out=o_re[bs_o], in_=sb_re)
                nc.gpsimd.dma_start(out=o_im[bs_o], in_=sb_im)
    nc.compile()
    return nc


def kernel(x, theta_1, theta_2, squeezing_r, displacement_r, kerr_params):
    global LAST_RESULT
    x = np.asarray(x, dtype=np.float32)
    UT = _build_UT(np.asarray(theta_1, np.float64), np.asarray(theta_2, np.float64),
                   np.asarray(squeezing_r, np.float64),
                   np.asarray(displacement_r, np.float64),
                   np.asarray(kerr_params, np.float64))
    psi0 = _build_psi0(x.astype(np.float64))          # (B, 729) complex128
    p_t = psi0.T                                      # (729, B)

    UT_pad = np.zeros((DIM_PAD, DIM), np.complex128)
    UT_pad[:DIM] = UT
    p_pad = np.zeros((DIM_PAD, BATCH), np.complex128)
    p_pad[:DIM] = p_t

    def tile_pm(arr):
        """[768, W] -> [256, 3*W]: half-blocks, each [128, 3*W] contiguous."""
        w = arr.shape[1]
        return np.ascontiguousarray(
            arr.reshape(2, 3, 128, w).transpose(0, 2, 1, 3).reshape(256, 3 * w),
            np.float32)

    in_maps = []
    for c in range(N_CORES):
        q, h = divmod(c, 2)
        bsl = slice(q * B_SHARD, (q + 1) * B_SHARD)
        isl = slice(I_START[h], I_START[h] + I_SHARD)
        in_maps.append({
            "u_re": tile_pm(UT_pad.real[:, isl]),
            "u_im": tile_pm(UT_pad.imag[:, isl]),
            "p_re": tile_pm(p_pad.real[:, bsl]),
            "p_im": tile_pm(p_pad.imag[:, bsl]),
        })

    # bass_utils' trace path does `from antenv.axon_hooks import ...`
    # unguarded; this image's antenv lacks that module.  Provide a stub so
    # tracing degrades gracefully instead of crashing (e.g. if BASS_TRACE=1).
    try:
        import antenv.axon_hooks  # noqa: F401
    except ImportError:
        import sys
        import types
        stub = types.ModuleType("antenv.axon_hooks")
        stub._hook = None
        stub.set_axon_ntff_profile_hook = lambda h: setattr(stub, "_hook", h)
        stub.get_axon_ntff_profile_hook = lambda: stub._hook
        sys.modules["antenv.axon_hooks"] = stub

    from concourse.bass_utils import run_bass_kernel_spmd
    nc = _build_bass()
    res = run_bass_kernel_spmd(nc, in_maps, core_ids=list(range(N_CORES)),
                               trace=bool(int(os.environ.get("KERNEL_TRACE", "0"))))
    LAST_RESULT = res

    psi = np.empty((BATCH, DIM), dtype=np.complex128)
    for c in range(N_CORES):
        q, h = divmod(c, 2)
        o = res.results[c]
        sh = (o["o_re"].astype(np.float64)
              + 1j * o["o_im"].astype(np.float64))
        bsl = slice(q * B_SHARD, (q + 1) * B_SHARD)
        if h == 0:
            psi[bsl, 0:I_SHARD] = sh
        else:
            psi[bsl, I_SHARD:DIM] = sh[:, I_SHARD - (DIM - I_SHARD):]
    return _expectation(psi).astype(np.float32)
